# revision 4
# baseline (speedup 1.0000x reference)
"""Pairwise KL divergence kernel for Trainium2, SPMD across 8 NeuronCores.

out[n, m] = sum_d a[n,d]*(log a[n,d] - log b[m,d])
          = ent[n] - (a @ log(b)^T)[n, m],  ent = rowsum(a * log a)

Sharding: a (and output rows) split 8 ways; b replicated.
Per core: a_shard (1024, 64), b (8192, 64) -> out_shard (1024, 8192).

Layout trick: b is loaded CONTIGUOUSLY (partition p holds rows 64p..64p+63,
4 KB descriptors instead of 256 B), so the PE transpose of C[:, r, :]
produces lb^T columns {64p + r} - an m-permuted lbT. The permutation is
undone for free in access patterns: the matmul's moving operand streams
(r, p) pairs so that psum column j' = 8r + p holds natural column 64p + r,
and the evac writes through a reordering AP. HBM out stays natural/dense.

Pipeline per core (fp16 GEMM operands, fp16 output staging):
  - issue a + 4 contiguous b-quarter DMAs up front.
  - a cast to fp16 on DVE; aT via 8 PE transposes (fp16, 1 cyc/row).
  - ent: la = Ln(a) [ACT], prod = a*la [DVE], per-tile reduce [DVE].
  - b: lb = Ln(b) -> fp16 [ACT, 4 pieces]; 64 PE transposes -> lbT_perm
    (64, 64, 128) fp16; PSUM->SBUF copies on DVE (contiguous, 2x mode).
  - GEMM: 8 n-tiles x 16 m-tiles: psum(128,512)fp32 = aT_t.T @ lbT_perm
    slice at 1 cyc/row. Evac 2 banks at a time fused with the entropy
    term (-psum + ent -> fp16), alternating ACT/DVE; half-tile (1 MB)
    DMAs, quarter-tile on the last n-tile to shorten the drain tail.
  - host upcasts fp16 -> fp32.

Precision: fp16 GEMM operands + fp16 output give ~5e-4 max rel err vs the
fp32 reference, well under the 2e-2 gate.
"""

import numpy as np

N, M, D = 8192, 8192, 64
NCORES = 8
NSHARD = N // NCORES          # 1024 rows of a per core
NT = NSHARD // 128            # 8 n-tiles per core
MT = M // 512                 # 16 m-tiles of 512
RT = 64                       # b transposes (one per r = row-within-partition)
R_CHUNK = 16                  # r values per Ln piece

# GEMM operand dtype: fp16/bf16 (1 cyc/row), fp32r (1 cyc/row, fp32 data),
# fp32 (4 cyc/row, exact)
MM_DTYPE = "fp16"
# output staging/DMA dtype: fp16 halves the dominant output traffic
OUT_DTYPE = "fp16"

_CACHE = {}


def _build(mm_dtype, out_dtype):
    from contextlib import ExitStack

    import concourse.bacc as bacc_mod
    import concourse.bass as bass
    import concourse.mybir as mybir
    import concourse.tile as tile
    from concourse.masks import make_identity

    FP32 = mybir.dt.float32
    AF = mybir.ActivationFunctionType
    ALU = mybir.AluOpType
    AX = mybir.AxisListType

    DT_MM = {
        "fp16": mybir.dt.float16,
        "bf16": mybir.dt.bfloat16,
        "fp32": FP32,
        "fp32r": mybir.dt.float32r,
    }[mm_dtype]
    DT_OUT = {"fp16": mybir.dt.float16, "fp32": FP32}[out_dtype]
    two_byte = mm_dtype in ("fp16", "bf16")
    TP_DT = DT_MM if two_byte else FP32

    nc = bacc_mod.Bacc()
    a_d = nc.dram_tensor("a", [NSHARD, D], FP32, kind="ExternalInput")
    b_d = nc.dram_tensor("b", [M, D], FP32, kind="ExternalInput")
    out_d = nc.dram_tensor("out", [NSHARD, M], DT_OUT, kind="ExternalOutput")

    n_pieces = RT // R_CHUNK

    with tile.TileContext(nc) as tc, ExitStack() as ctx:
        consts = ctx.enter_context(tc.tile_pool(name="consts", bufs=1))
        apool = ctx.enter_context(tc.tile_pool(name="apool", bufs=1))
        bpool = ctx.enter_context(tc.tile_pool(name="bpool", bufs=1))
        lbtp = ctx.enter_context(tc.tile_pool(name="lbtp", bufs=1))
        tpsum = ctx.enter_context(tc.tile_pool(name="tpsum", bufs=2, space="PSUM"))
        mmps = ctx.enter_context(tc.tile_pool(name="mmps", bufs=3, space="PSUM"))
        stage = ctx.enter_context(tc.tile_pool(name="stage", bufs=2))

        ident = consts.tile([128, 128], TP_DT)
        make_identity(nc, ident)
        # Dummy transpose so PE observes the gpsimd (ident) sem here: the
        # matmul/LDW struct only carries ONE sync wait, so later transposes
        # must each need at most one sem (codegen: "Too many sync waits").
        warm = tpsum.tile([128, 128], TP_DT, tag="tp")
        nc.tensor.transpose(warm, ident, ident)

        # ---------------- input DMAs, all issued up front ----------------
        a_nat = apool.tile([128, NT, D], FP32)        # a row t*128+p at [p, t, :]
        nc.sync.dma_start(out=a_nat, in_=a_d[:, :].rearrange("(t p) d -> p t d", p=128))
        # contiguous b: partition p holds rows 64p..64p+63 (4 KB descriptors)
        c_nat = bpool.tile([128, RT, D], FP32)        # b row 64p+r at [p, r, :]
        b_r = b_d[:, :].rearrange("(p r) d -> p r d", p=128)
        for h in range(n_pieces):
            sl = slice(h * R_CHUNK, (h + 1) * R_CHUNK)
            nc.sync.dma_start(out=c_nat[:, sl, :], in_=b_r[:, sl, :])

        # ---------------- a prologue ----------------
        if two_byte:
            a_mm = apool.tile([128, NT, D], DT_MM)
            nc.vector.tensor_copy(a_mm, a_nat)        # DVE: ACT stays free for Ln
        else:
            a_mm = a_nat
        aT = apool.tile([64, NT, 128], DT_MM)         # aT[:, t, :] = a tile t transposed
        for g in range(2):
            tp = tpsum.tile([64, 4, 128], TP_DT, tag="tp")
            for j in range(4):
                nc.tensor.transpose(tp[:, j], a_mm[:, g * 4 + j, :], ident)
            nc.vector.tensor_copy(aT[:, g * 4 : (g + 1) * 4, :], tp)

        # ---------------- b prologue ----------------
        # lbT_perm[d, r, p] = log(b)[64p + r, d]
        lbT = lbtp.tile([64, RT, 128], DT_MM)
        lb = bpool.tile([128, RT, D], TP_DT)

        def b_piece(h):
            sl = slice(h * R_CHUNK, (h + 1) * R_CHUNK)
            nc.scalar.activation(lb[:, sl, :], c_nat[:, sl, :], AF.Ln)
            for gg in range(R_CHUNK // 4):
                r0 = h * R_CHUNK + gg * 4
                tp = tpsum.tile([64, 4, 128], TP_DT, tag="tp")
                for j in range(4):
                    nc.tensor.transpose(tp[:, j], lb[:, r0 + j, :], ident)
                nc.vector.tensor_copy(lbT[:, r0 : r0 + 4, :], tp)

        b_piece(0)

        # entropy term (needed by the first evac, after piece 0 in ACT order)
        la = apool.tile([128, NT, D], FP32)
        nc.scalar.activation(la, a_nat, AF.Ln)
        prod = apool.tile([128, NT, D], FP32)
        nc.vector.tensor_mul(prod, a_nat, la)
        ent = apool.tile([128, NT], FP32)
        for t in range(NT):
            nc.vector.reduce_sum(ent[:, t : t + 1], prod[:, t, :], axis=AX.X)

        for h in range(1, n_pieces):
            b_piece(h)

        # ---------------- main GEMM + fused evac ----------------
        # moving operand for m-tile mi: (r, p) with p in [8mi, 8mi+8);
        # psum col j' = 8r + p-8mi holds natural col 64p + r.
        out_r = out_d[:, :].rearrange("(t p) (c m) -> t p c m", p=128, m=512)
        for t in range(NT):
            out_sb = stage.tile([128, MT, 512], DT_OUT, tag="out_sb")
            lhsT = aT[:, t, :]
            ent_t = ent[:, t : t + 1]
            dma_q = MT // 4 if t == NT - 1 else MT // 2
            for g in range(MT // 2):
                ps = mmps.tile([128, 2, 512], FP32, tag="ps")
                for j in range(2):
                    mi = g * 2 + j
                    nc.tensor.matmul(
                        ps[:, j],
                        lhsT,
                        lbT[:, :, mi * 8 : (mi + 1) * 8],
                        start=True,
                        stop=True,
                    )
                # unpermute in the evac write AP: psum (r, q) -> out col 64q+r
                src = ps.rearrange("p c (r q) -> p c r q", q=8)
                dst = out_sb[:, g * 2 : (g + 1) * 2, :].rearrange(
                    "p c (q r) -> p c r q", q=8
                )
                if g % 2 == 0:
                    nc.scalar.activation(dst, src, AF.Identity, bias=ent_t, scale=-1.0)
                else:
                    nc.vector.tensor_scalar(dst, src, -1.0, ent_t, ALU.mult, ALU.add)
                done = (g + 1) * 2
                if done % dma_q == 0:
                    c0 = done - dma_q
                    nc.sync.dma_start(
                        out=out_r[t, :, c0:done, :],
                        in_=out_sb[:, c0:done, :],
                    )
    # bacc lowering: splits multi-sem waits onto event-semaphore/nop
    # instructions (HW allows one sync wait per engine instruction).
    nc.compile()
    return nc


def _run(a, b, trace=False):
    from concourse.bass_utils import run_bass_kernel_spmd

    key = (MM_DTYPE, OUT_DTYPE)
    if key not in _CACHE:
        _CACHE[key] = _build(*key)
    nc = _CACHE[key]
    a = np.ascontiguousarray(np.asarray(a, dtype=np.float32))
    b = np.ascontiguousarray(np.asarray(b, dtype=np.float32))
    in_maps = [
        {"a": a[i * NSHARD : (i + 1) * NSHARD], "b": b} for i in range(NCORES)
    ]
    res = run_bass_kernel_spmd(nc, in_maps, list(range(NCORES)), trace=trace)
    out = np.concatenate(
        [np.asarray(r["out"], dtype=np.float32) for r in res.results], axis=0
    )
    return out, res


def kernel(a, b):
    out, _ = _run(a, b, trace=False)
    return out


# revision 7
# speedup vs baseline: 1.7453x; 1.7453x over previous
"""Pairwise KL divergence kernel for Trainium2, SPMD across 8 NeuronCores.

out[n, m] = sum_d a[n,d]*(log a[n,d] - log b[m,d])
          = ent[n] - (a @ log(b)^T)[n, m],  ent = rowsum(a * log a)

Sharding: a (and output rows) split 8 ways; b replicated.
Per core: a_shard (1024, 64), b (8192, 64) -> out_shard (1024, 8192).

Structure (fp16 GEMM operands, fp16 output staging), m-chunk-major:
  - input DMAs issued immediately, split across the two HWDGE sequencers
    (sync: a + b chunks 0-1, scalar: b chunks 2-3) so descriptor
    generation for the strided (256 B/row) b loads runs in parallel.
  - a cast to fp16 on DVE; aT via 8 PE transposes; ent chain (Ln(a) [ACT],
    a*la [DVE], reduce [DVE]) ordered so it hides under the b0 load.
  - main loop is M-CHUNK-MAJOR: for each b chunk h (16 m-tiles' worth of
    lbT): 16 PE transposes -> lbT chunk, then 8 n-tiles x 4 m-tiles of
    GEMM + fused evac (-psum + ent -> fp16, alternating ACT/DVE), one
    512 KB output strip DMA per (chunk, n-tile). PE alternates compact
    transpose blocks (1.7 us) with dense matmul blocks (14 us) and never
    starves; Ln(b h+1) is issued ahead of chunk h's evacs in the ACT queue.
  - host upcasts fp16 -> fp32.

Precision: fp16 GEMM operands + fp16 output give ~5e-4 max rel err vs the
fp32 reference, well under the 2e-2 gate.
"""

import numpy as np

N, M, D = 8192, 8192, 64
NCORES = 8
NSHARD = N // NCORES          # 1024 rows of a per core
NT = NSHARD // 128            # 8 n-tiles per core
MT = M // 512                 # 16 m-tiles of 512
BT = M // 128                 # 64 b row-tiles to transpose
B_CHUNK = 16                  # b row-tiles per chunk (2048 rows, 4 m-tiles)
N_CHUNKS = BT // B_CHUNK      # 4

MM_DTYPE = "fp16"
OUT_DTYPE = "fp16"

_CACHE = {}


def _build(mm_dtype, out_dtype):
    from contextlib import ExitStack

    import concourse.bacc as bacc_mod
    import concourse.bass as bass
    import concourse.mybir as mybir
    import concourse.tile as tile
    from concourse.masks import make_identity

    FP32 = mybir.dt.float32
    AF = mybir.ActivationFunctionType
    ALU = mybir.AluOpType
    AX = mybir.AxisListType

    DT_MM = {
        "fp16": mybir.dt.float16,
        "bf16": mybir.dt.bfloat16,
        "fp32": FP32,
        "fp32r": mybir.dt.float32r,
    }[mm_dtype]
    DT_OUT = {"fp16": mybir.dt.float16, "fp32": FP32}[out_dtype]
    two_byte = mm_dtype in ("fp16", "bf16")
    TP_DT = DT_MM if two_byte else FP32

    nc = bacc_mod.Bacc()
    a_d = nc.dram_tensor("a", [NSHARD, D], FP32, kind="ExternalInput")
    b_d = nc.dram_tensor("b", [M, D], FP32, kind="ExternalInput")
    out_d = nc.dram_tensor("out", [NSHARD, M], DT_OUT, kind="ExternalOutput")

    with tile.TileContext(nc) as tc, ExitStack() as ctx:
        consts = ctx.enter_context(tc.tile_pool(name="consts", bufs=1))
        apool = ctx.enter_context(tc.tile_pool(name="apool", bufs=1))
        bpool = ctx.enter_context(tc.tile_pool(name="bpool", bufs=N_CHUNKS))
        lbpool = ctx.enter_context(tc.tile_pool(name="lbpool", bufs=2))
        lbtp = ctx.enter_context(tc.tile_pool(name="lbtp", bufs=1))
        tpsum = ctx.enter_context(tc.tile_pool(name="tpsum", bufs=2, space="PSUM"))
        mmps = ctx.enter_context(tc.tile_pool(name="mmps", bufs=3, space="PSUM"))
        stage = ctx.enter_context(tc.tile_pool(name="stage", bufs=2))

        ident = consts.tile([128, 128], TP_DT)
        make_identity(nc, ident)
        # Dummy transpose so PE observes the gpsimd (ident) sem here: the
        # matmul/LDW struct only carries ONE sync wait, so later transposes
        # must each need at most one sem (codegen: "Too many sync waits").
        warm = tpsum.tile([128, 128], TP_DT, tag="tp")
        nc.tensor.transpose(warm, ident, ident)

        # -------- input DMAs: split across the two HWDGE sequencers --------
        a_nat = apool.tile([128, NT, D], FP32)        # a row t*128+p at [p, t, :]
        nc.sync.dma_start(out=a_nat, in_=a_d[:, :].rearrange("(t p) d -> p t d", p=128))
        b_r = b_d[:, :].rearrange("(t p) d -> p t d", p=128)
        b_nats = []
        for h in range(N_CHUNKS):
            b_nat = bpool.tile([128, B_CHUNK, D], FP32, tag="b_nat")
            eng = nc.sync if h < N_CHUNKS // 2 else nc.scalar
            eng.dma_start(out=b_nat, in_=b_r[:, h * B_CHUNK : (h + 1) * B_CHUNK, :])
            b_nats.append(b_nat)

        # ---------------- a prologue ----------------
        if two_byte:
            a_mm = apool.tile([128, NT, D], DT_MM)
            nc.vector.tensor_copy(a_mm, a_nat)        # DVE: ACT stays free for Ln
        else:
            a_mm = a_nat
        aT = apool.tile([64, NT, 128], DT_MM)         # aT[:, t, :] = a tile t transposed
        for g in range(2):
            tp = tpsum.tile([64, 4, 128], TP_DT, tag="tp")
            for j in range(4):
                nc.tensor.transpose(tp[:, j], a_mm[:, g * 4 + j, :], ident)
            nc.vector.tensor_copy(aT[:, g * 4 : (g + 1) * 4, :], tp)

        lbT = lbtp.tile([64, BT, 128], DT_MM)         # lbT[:, bt, :] = lb tile bt transposed

        def b_ln(h, split=1):
            """Ln(b chunk h) -> fp16, optionally in `split` pieces."""
            lb = lbpool.tile([128, B_CHUNK, D], TP_DT, tag="lb")
            step = B_CHUNK // split
            for s in range(split):
                sl = slice(s * step, (s + 1) * step)
                nc.scalar.activation(lb[:, sl, :], b_nats[h][:, sl, :], AF.Ln)
            return lb

        def b_transpose(h, lb):
            for gg in range(B_CHUNK // 4):
                bt0 = h * B_CHUNK + gg * 4
                tp = tpsum.tile([64, 4, 128], TP_DT, tag="tp")
                for j in range(4):
                    nc.tensor.transpose(tp[:, j], lb[:, gg * 4 + j, :], ident)
                nc.vector.tensor_copy(lbT[:, bt0 : bt0 + 4, :], tp)

        # chunk 0: split Ln so the first transposes start as early as possible
        lb0 = b_ln(0, split=2)

        # entropy chain (ent needed by the first evac; hides under b0 load)
        la = apool.tile([128, NT, D], FP32)
        nc.scalar.activation(la, a_nat, AF.Ln)
        prod = apool.tile([128, NT, D], FP32)
        nc.vector.tensor_mul(prod, a_nat, la)
        ent = apool.tile([128, NT], FP32)
        for t in range(NT):
            nc.vector.reduce_sum(ent[:, t : t + 1], prod[:, t, :], axis=AX.X)

        b_transpose(0, lb0)

        # ---------------- main loop: m-chunk-major ----------------
        # chunk h covers m in [2048h, 2048h+2048): 4 m-tiles x 8 n-tiles
        MT_C = MT // N_CHUNKS                         # 4 m-tiles per chunk
        out_r = out_d[:, :].rearrange(
            "(t p) (h c m) -> t p h c m", p=128, h=N_CHUNKS, m=512
        )
        for h in range(N_CHUNKS):
            if h + 1 < N_CHUNKS:
                # next chunk's Ln goes ahead of this chunk's evacs in ACT FIFO
                lb_next = b_ln(h + 1)
            for t in range(NT):
                out_sb = stage.tile([128, MT_C, 512], DT_OUT, tag="out_sb")
                lhsT = aT[:, t, :]
                ent_t = ent[:, t : t + 1]
                for g in range(MT_C // 2):
                    ps = mmps.tile([128, 2, 512], FP32, tag="ps")
                    for j in range(2):
                        mi = h * MT_C + g * 2 + j
                        nc.tensor.matmul(
                            ps[:, j],
                            lhsT,
                            lbT[:, mi * 4 : (mi + 1) * 4, :],
                            start=True,
                            stop=True,
                        )
                    dst = out_sb[:, g * 2 : (g + 1) * 2, :]
                    if (t + g) % 2 == 0:
                        nc.scalar.activation(
                            dst, ps, AF.Identity, bias=ent_t, scale=-1.0
                        )
                    else:
                        nc.vector.tensor_scalar(
                            dst, ps, -1.0, ent_t, ALU.mult, ALU.add
                        )
                nc.sync.dma_start(out=out_r[t, :, h, :, :], in_=out_sb)
            if h + 1 < N_CHUNKS:
                b_transpose(h + 1, lb_next)
    # bacc lowering: splits multi-sem waits onto event-semaphore/nop
    # instructions (HW allows one sync wait per engine instruction).
    nc.compile()
    return nc


def _run(a, b, trace=False):
    from concourse.bass_utils import run_bass_kernel_spmd

    key = (MM_DTYPE, OUT_DTYPE)
    if key not in _CACHE:
        _CACHE[key] = _build(*key)
    nc = _CACHE[key]
    a = np.ascontiguousarray(np.asarray(a, dtype=np.float32))
    b = np.ascontiguousarray(np.asarray(b, dtype=np.float32))
    in_maps = [
        {"a": a[i * NSHARD : (i + 1) * NSHARD], "b": b} for i in range(NCORES)
    ]
    res = run_bass_kernel_spmd(nc, in_maps, list(range(NCORES)), trace=trace)
    out = np.concatenate(
        [np.asarray(r["out"], dtype=np.float32) for r in res.results], axis=0
    )
    return out, res


def kernel(a, b):
    out, _ = _run(a, b, trace=False)
    return out


# revision 9
# speedup vs baseline: 1.9669x; 1.1270x over previous
"""Pairwise KL divergence kernel for Trainium2, SPMD across 8 NeuronCores.

out[n, m] = sum_d a[n,d]*(log a[n,d] - log b[m,d])
          = ent[n] - (a @ log(b)^T)[n, m],  ent = rowsum(a * log a)

Sharding: a (and output rows) split 8 ways; b replicated.
Per core: a_shard (1024, 64), b (8192, 64) -> out_shard (1024, 8192).

The PE clock is pinned ~1.2 GHz in this environment (no HAM ramp), so the
fp16 GEMM (128 x 512-row matmuls ~427 ns each) plus 73 transposes ~62 us
of PE time is the span floor; the structure keeps PE saturated and hides
everything else under it:
  - all input DMAs issued up front on the sync sequencer; chunk 0 of b is
    loaded/Ln'd in halves so the first transposes (and first main matmuls)
    start as early as possible.
  - a cast to fp16 on DVE; aT via 8 PE transposes; ent chain (Ln(a) [ACT]
    after Ln(b0), a*la + reduce [DVE]) completes before the first evac.
  - b: lb = Ln(b) -> fp16 [ACT]; 64 PE transposes -> lbT; PSUM->SBUF
    copies on DVE for chunk 0, on GPSIMD for chunks 1-3 (keeps the DVE
    FIFO free for main-loop evacs - no head-of-line blocking).
  - main loop n-tile-major: 16 m-tiles per n-tile; psum(128,512)fp32 =
    aT_t.T @ lbT tile; evac 2 banks at a time fused with the entropy term
    (-psum + ent -> fp16), DVE on even groups (first evac has no ACT-queue
    latency), ACT on odd; half-tile 1 MB DMAs, quarter-tile on the last
    n-tile to shorten the drain tail.
  - host upcasts fp16 -> fp32.

Precision: fp16 GEMM operands + fp16 output give ~5e-4 max rel err vs the
fp32 reference, well under the 2e-2 gate.
"""

import numpy as np

N, M, D = 8192, 8192, 64
NCORES = 8
NSHARD = N // NCORES          # 1024 rows of a per core
NT = NSHARD // 128            # 8 n-tiles per core
MT = M // 512                 # 16 m-tiles of 512
BT = M // 128                 # 64 b row-tiles to transpose
B_CHUNK = 16                  # b row-tiles per chunk (2048 rows)
N_CHUNKS = BT // B_CHUNK      # 4

MM_DTYPE = "fp16"
OUT_DTYPE = "fp16"

_CACHE = {}


def _build(mm_dtype, out_dtype):
    from contextlib import ExitStack

    import concourse.bacc as bacc_mod
    import concourse.bass as bass
    import concourse.mybir as mybir
    import concourse.tile as tile
    from concourse.masks import make_identity

    FP32 = mybir.dt.float32
    AF = mybir.ActivationFunctionType
    ALU = mybir.AluOpType
    AX = mybir.AxisListType

    DT_MM = {
        "fp16": mybir.dt.float16,
        "bf16": mybir.dt.bfloat16,
        "fp32": FP32,
        "fp32r": mybir.dt.float32r,
    }[mm_dtype]
    DT_OUT = {"fp16": mybir.dt.float16, "fp32": FP32}[out_dtype]
    two_byte = mm_dtype in ("fp16", "bf16")
    TP_DT = DT_MM if two_byte else FP32

    nc = bacc_mod.Bacc()
    a_d = nc.dram_tensor("a", [NSHARD, D], FP32, kind="ExternalInput")
    b_d = nc.dram_tensor("b", [M, D], FP32, kind="ExternalInput")
    out_d = nc.dram_tensor("out", [NSHARD, M], DT_OUT, kind="ExternalOutput")

    with tile.TileContext(nc) as tc, ExitStack() as ctx:
        consts = ctx.enter_context(tc.tile_pool(name="consts", bufs=1))
        apool = ctx.enter_context(tc.tile_pool(name="apool", bufs=1))
        bpool = ctx.enter_context(tc.tile_pool(name="bpool", bufs=N_CHUNKS))
        lbpool = ctx.enter_context(tc.tile_pool(name="lbpool", bufs=2))
        lbtp = ctx.enter_context(tc.tile_pool(name="lbtp", bufs=1))
        tpsum = ctx.enter_context(tc.tile_pool(name="tpsum", bufs=2, space="PSUM"))
        mmps = ctx.enter_context(tc.tile_pool(name="mmps", bufs=3, space="PSUM"))
        stage = ctx.enter_context(tc.tile_pool(name="stage", bufs=2))

        ident = consts.tile([128, 128], TP_DT)
        make_identity(nc, ident)
        # Dummy transpose so PE observes the gpsimd (ident) sem here: the
        # matmul/LDW struct only carries ONE sync wait, so later transposes
        # must each need at most one sem (codegen: "Too many sync waits").
        warm = tpsum.tile([128, 128], TP_DT, tag="tp")
        nc.tensor.transpose(warm, ident, ident)

        # -------- input DMAs, all issued up front on the sync sequencer ----
        a_nat = apool.tile([128, NT, D], FP32)        # a row t*128+p at [p, t, :]
        nc.sync.dma_start(out=a_nat, in_=a_d[:, :].rearrange("(t p) d -> p t d", p=128))
        b_r = b_d[:, :].rearrange("(t p) d -> p t d", p=128)
        b_nats = []
        for h in range(N_CHUNKS):
            b_nat = bpool.tile([128, B_CHUNK, D], FP32, tag="b_nat")
            if h == 0:
                # split so the first half's Ln can start earliest
                half = B_CHUNK // 2
                nc.sync.dma_start(out=b_nat[:, :half, :], in_=b_r[:, :half, :])
                nc.sync.dma_start(
                    out=b_nat[:, half:, :], in_=b_r[:, half:B_CHUNK, :]
                )
            else:
                nc.sync.dma_start(
                    out=b_nat, in_=b_r[:, h * B_CHUNK : (h + 1) * B_CHUNK, :]
                )
            b_nats.append(b_nat)

        # ---------------- a prologue ----------------
        if two_byte:
            a_mm = apool.tile([128, NT, D], DT_MM)
            nc.vector.tensor_copy(a_mm, a_nat)        # DVE: ACT stays free for Ln
        else:
            a_mm = a_nat
        aT = apool.tile([64, NT, 128], DT_MM)         # aT[:, t, :] = a tile t transposed
        for g in range(2):
            tp = tpsum.tile([64, 4, 128], TP_DT, tag="tp")
            for j in range(4):
                nc.tensor.transpose(tp[:, j], a_mm[:, g * 4 + j, :], ident)
            nc.vector.tensor_copy(aT[:, g * 4 : (g + 1) * 4, :], tp)

        lbT = lbtp.tile([64, BT, 128], DT_MM)         # lbT[:, bt, :] = lb tile bt transposed

        def b_chunk(h, split=1, copy_eng=None):
            lb = lbpool.tile([128, B_CHUNK, D], TP_DT, tag="lb")
            step = B_CHUNK // split
            for s in range(split):
                sl = slice(s * step, (s + 1) * step)
                nc.scalar.activation(lb[:, sl, :], b_nats[h][:, sl, :], AF.Ln)
            for gg in range(B_CHUNK // 4):
                bt0 = h * B_CHUNK + gg * 4
                tp = tpsum.tile([64, 4, 128], TP_DT, tag="tp")
                for j in range(4):
                    nc.tensor.transpose(tp[:, j], lb[:, gg * 4 + j, :], ident)
                copy_eng(lbT[:, bt0 : bt0 + 4, :], tp)

        b_chunk(0, split=2, copy_eng=nc.vector.tensor_copy)

        # entropy chain (needed by the first evac, hides under the b loads)
        la = apool.tile([128, NT, D], FP32)
        nc.scalar.activation(la, a_nat, AF.Ln)
        prod = apool.tile([128, NT, D], FP32)
        nc.vector.tensor_mul(prod, a_nat, la)
        ent = apool.tile([128, NT], FP32)
        for t in range(NT):
            nc.vector.reduce_sum(ent[:, t : t + 1], prod[:, t, :], axis=AX.X)

        # all Ln's were issued early, so chunks 1-3's transposes (and these
        # DVE copies) complete before the first evacs need the DVE FIFO
        for h in range(1, N_CHUNKS):
            b_chunk(h, copy_eng=nc.vector.tensor_copy)

        # ---------------- main GEMM + fused evac ----------------
        out_r = out_d[:, :].rearrange("(t p) (c m) -> t p c m", p=128, m=512)
        for t in range(NT):
            out_sb = stage.tile([128, MT, 512], DT_OUT, tag="out_sb")
            lhsT = aT[:, t, :]
            ent_t = ent[:, t : t + 1]
            dma_q = MT // 4 if t == NT - 1 else MT // 2
            for g in range(MT // 2):
                ps = mmps.tile([128, 2, 512], FP32, tag="ps")
                for j in range(2):
                    mi = g * 2 + j
                    nc.tensor.matmul(
                        ps[:, j],
                        lhsT,
                        lbT[:, mi * 4 : (mi + 1) * 4, :],
                        start=True,
                        stop=True,
                    )
                dst = out_sb[:, g * 2 : (g + 1) * 2, :]
                if g % 2 == 0:
                    # DVE first: the ACT queue is still draining Ln's early on
                    nc.vector.tensor_scalar(dst, ps, -1.0, ent_t, ALU.mult, ALU.add)
                else:
                    nc.scalar.activation(dst, ps, AF.Identity, bias=ent_t, scale=-1.0)
                done = (g + 1) * 2
                if done % dma_q == 0:
                    c0 = done - dma_q
                    nc.sync.dma_start(
                        out=out_r[t, :, c0:done, :],
                        in_=out_sb[:, c0:done, :],
                    )
    # bacc lowering: splits multi-sem waits onto event-semaphore/nop
    # instructions (HW allows one sync wait per engine instruction).
    nc.compile()
    return nc


def _run(a, b, trace=False):
    from concourse.bass_utils import run_bass_kernel_spmd

    key = (MM_DTYPE, OUT_DTYPE)
    if key not in _CACHE:
        _CACHE[key] = _build(*key)
    nc = _CACHE[key]
    a = np.ascontiguousarray(np.asarray(a, dtype=np.float32))
    b = np.ascontiguousarray(np.asarray(b, dtype=np.float32))
    in_maps = [
        {"a": a[i * NSHARD : (i + 1) * NSHARD], "b": b} for i in range(NCORES)
    ]
    res = run_bass_kernel_spmd(nc, in_maps, list(range(NCORES)), trace=trace)
    out = np.concatenate(
        [np.asarray(r["out"], dtype=np.float32) for r in res.results], axis=0
    )
    return out, res


def kernel(a, b):
    out, _ = _run(a, b, trace=False)
    return out


# revision 12
# speedup vs baseline: 2.1157x; 1.0757x over previous
"""Pairwise KL divergence kernel for Trainium2, SPMD across 8 NeuronCores.

out[n, m] = sum_d a[n,d]*(log a[n,d] - log b[m,d])
          = ent[n] - (a @ log(b)^T)[n, m],  ent = rowsum(a * log a)

Sharding: a (and output rows) split 8 ways; b replicated.
Per core: a_shard (1024, 64), b (8192, 64) -> out_shard (1024, 8192).

The PE clock is pinned ~1.2 GHz in this environment (no HAM ramp), so the
fp16 GEMM (128 x 512-row matmuls ~427 ns each) plus 73 transposes ~62 us
of PE time is the span floor; the structure keeps PE saturated and hides
everything else under it:
  - all input DMAs issued up front on the sync sequencer; chunk 0 of b is
    loaded/Ln'd in halves so the first transposes (and first main matmuls)
    start as early as possible.
  - a cast to fp16 on DVE; aT via 8 PE transposes; ent chain (Ln(a) [ACT]
    after Ln(b0), a*la + reduce [DVE]) completes before the first evac.
  - b: lb = Ln(b) -> fp16 [ACT]; 64 PE transposes -> lbT; PSUM->SBUF
    copies on DVE for chunk 0, on GPSIMD for chunks 1-3 (keeps the DVE
    FIFO free for main-loop evacs - no head-of-line blocking).
  - main loop n-tile-major: 16 m-tiles per n-tile; psum(128,512)fp32 =
    aT_t.T @ lbT tile; evac 2 banks at a time fused with the entropy term
    (-psum + ent -> fp16), DVE on even groups (first evac has no ACT-queue
    latency), ACT on odd; half-tile 1 MB DMAs, quarter-tile on the last
    n-tile to shorten the drain tail.
  - host upcasts fp16 -> fp32.

Precision: fp16 GEMM operands + fp16 output give ~5e-4 max rel err vs the
fp32 reference, well under the 2e-2 gate.
"""

import numpy as np

N, M, D = 8192, 8192, 64
NCORES = 8
NSHARD = N // NCORES          # 1024 rows of a per core
NT = NSHARD // 128            # 8 n-tiles per core
MT = M // 512                 # 16 m-tiles of 512
BT = M // 128                 # 64 b row-tiles to transpose
B_CHUNK = 16                  # b row-tiles per chunk (2048 rows)
N_CHUNKS = BT // B_CHUNK      # 4

MM_DTYPE = "fp16"
OUT_DTYPE = "fp16"

_CACHE = {}


def _build(mm_dtype, out_dtype):
    from contextlib import ExitStack

    import concourse.bacc as bacc_mod
    import concourse.bass as bass
    import concourse.mybir as mybir
    import concourse.tile as tile
    from concourse.masks import make_identity

    FP32 = mybir.dt.float32
    AF = mybir.ActivationFunctionType
    ALU = mybir.AluOpType
    AX = mybir.AxisListType

    DT_MM = {
        "fp16": mybir.dt.float16,
        "bf16": mybir.dt.bfloat16,
        "fp32": FP32,
        "fp32r": mybir.dt.float32r,
    }[mm_dtype]
    DT_OUT = {"fp16": mybir.dt.float16, "fp32": FP32}[out_dtype]
    two_byte = mm_dtype in ("fp16", "bf16")
    TP_DT = DT_MM if two_byte else FP32

    nc = bacc_mod.Bacc()
    a_d = nc.dram_tensor("a", [NSHARD, D], FP32, kind="ExternalInput")
    b_d = nc.dram_tensor("b", [M, D], FP32, kind="ExternalInput")
    out_d = nc.dram_tensor("out", [NSHARD, M], DT_OUT, kind="ExternalOutput")

    with tile.TileContext(nc) as tc, ExitStack() as ctx:
        consts = ctx.enter_context(tc.tile_pool(name="consts", bufs=1))
        apool = ctx.enter_context(tc.tile_pool(name="apool", bufs=1))
        bpool = ctx.enter_context(tc.tile_pool(name="bpool", bufs=N_CHUNKS))
        lbpool = ctx.enter_context(tc.tile_pool(name="lbpool", bufs=2))
        lbtp = ctx.enter_context(tc.tile_pool(name="lbtp", bufs=1))
        tpsum = ctx.enter_context(tc.tile_pool(name="tpsum", bufs=2, space="PSUM"))
        mmps = ctx.enter_context(tc.tile_pool(name="mmps", bufs=3, space="PSUM"))
        stage = ctx.enter_context(tc.tile_pool(name="stage", bufs=3))

        ident = consts.tile([128, 128], TP_DT)
        make_identity(nc, ident)
        # Dummy transpose so PE observes the gpsimd (ident) sem here: the
        # matmul/LDW struct only carries ONE sync wait, so later transposes
        # must each need at most one sem (codegen: "Too many sync waits").
        warm = tpsum.tile([128, 128], TP_DT, tag="tp")
        nc.tensor.transpose(warm, ident, ident)

        # -------- input DMAs, all issued up front on the sync sequencer ----
        # b chunk 0 goes first (in halves): the earliest PE work is its
        # transposes, so its data must land before a's
        b_r = b_d[:, :].rearrange("(t p) d -> p t d", p=128)
        b_nats = []
        for h in range(N_CHUNKS):
            b_nat = bpool.tile([128, B_CHUNK, D], FP32, tag="b_nat")
            b_nats.append(b_nat)
        half = B_CHUNK // 2
        nc.sync.dma_start(out=b_nats[0][:, :half, :], in_=b_r[:, :half, :])
        nc.sync.dma_start(out=b_nats[0][:, half:, :], in_=b_r[:, half:B_CHUNK, :])
        a_nat = apool.tile([128, NT, D], FP32)        # a row t*128+p at [p, t, :]
        nc.sync.dma_start(out=a_nat, in_=a_d[:, :].rearrange("(t p) d -> p t d", p=128))
        for h in range(1, N_CHUNKS):
            nc.sync.dma_start(
                out=b_nats[h], in_=b_r[:, h * B_CHUNK : (h + 1) * B_CHUNK, :]
            )

        lbT = lbtp.tile([64, BT, 128], DT_MM)         # lbT[:, bt, :] = lb tile bt transposed

        def b_chunk(h, split=1, copy_eng=None):
            lb = lbpool.tile([128, B_CHUNK, D], TP_DT, tag="lb")
            step = B_CHUNK // split
            for s in range(split):
                sl = slice(s * step, (s + 1) * step)
                nc.scalar.activation(lb[:, sl, :], b_nats[h][:, sl, :], AF.Ln)
            for gg in range(B_CHUNK // 4):
                bt0 = h * B_CHUNK + gg * 4
                tp = tpsum.tile([64, 4, 128], TP_DT, tag="tp")
                for j in range(4):
                    nc.tensor.transpose(tp[:, j], lb[:, gg * 4 + j, :], ident)
                copy_eng(lbT[:, bt0 : bt0 + 4, :], tp)

        b_chunk(0, split=2, copy_eng=nc.vector.tensor_copy)

        # ---------------- a prologue (after chunk 0 in the PE FIFO: its
        # data lands later than b0's, and the PE wait queue is only 4 deep)
        if two_byte:
            a_mm = apool.tile([128, NT, D], DT_MM)
            nc.vector.tensor_copy(a_mm, a_nat)        # DVE: ACT stays free for Ln
        else:
            a_mm = a_nat
        aT = apool.tile([64, NT, 128], DT_MM)         # aT[:, t, :] = a tile t transposed
        for g in range(2):
            tp = tpsum.tile([64, 4, 128], TP_DT, tag="tp")
            for j in range(4):
                nc.tensor.transpose(tp[:, j], a_mm[:, g * 4 + j, :], ident)
            nc.vector.tensor_copy(aT[:, g * 4 : (g + 1) * 4, :], tp)

        # entropy chain (needed by the first evac, hides under the b loads)
        la = apool.tile([128, NT, D], FP32)
        nc.scalar.activation(la, a_nat, AF.Ln)
        prod = apool.tile([128, NT, D], FP32)
        nc.vector.tensor_mul(prod, a_nat, la)
        ent = apool.tile([128, NT], FP32)
        for t in range(NT):
            nc.vector.reduce_sum(ent[:, t : t + 1], prod[:, t, :], axis=AX.X)

        # all Ln's were issued early, so chunks 1-3's transposes (and these
        # DVE copies) complete before the first evacs need the DVE FIFO
        for h in range(1, N_CHUNKS):
            b_chunk(h, copy_eng=nc.vector.tensor_copy)

        # ---------------- main GEMM + fused evac ----------------
        out_r = out_d[:, :].rearrange("(t p) (c m) -> t p c m", p=128, m=512)
        for t in range(NT):
            out_sb = stage.tile([128, MT, 512], DT_OUT, tag="out_sb")
            lhsT = aT[:, t, :]
            ent_t = ent[:, t : t + 1]
            dma_q = MT // 4 if t == NT - 1 else MT // 2
            for g in range(MT // 2):
                ps = mmps.tile([128, 2, 512], FP32, tag="ps")
                for j in range(2):
                    mi = g * 2 + j
                    nc.tensor.matmul(
                        ps[:, j],
                        lhsT,
                        lbT[:, mi * 4 : (mi + 1) * 4, :],
                        start=True,
                        stop=True,
                    )
                dst = out_sb[:, g * 2 : (g + 1) * 2, :]
                if g % 2 == 0:
                    # DVE first: the ACT queue is still draining Ln's early on
                    nc.vector.tensor_scalar(dst, ps, -1.0, ent_t, ALU.mult, ALU.add)
                else:
                    nc.scalar.activation(dst, ps, AF.Identity, bias=ent_t, scale=-1.0)
                done = (g + 1) * 2
                if done % dma_q == 0:
                    c0 = done - dma_q
                    nc.sync.dma_start(
                        out=out_r[t, :, c0:done, :],
                        in_=out_sb[:, c0:done, :],
                    )
    # bacc lowering: splits multi-sem waits onto event-semaphore/nop
    # instructions (HW allows one sync wait per engine instruction).
    nc.compile()
    return nc


def _run(a, b, trace=False):
    from concourse.bass_utils import run_bass_kernel_spmd

    key = (MM_DTYPE, OUT_DTYPE)
    if key not in _CACHE:
        _CACHE[key] = _build(*key)
    nc = _CACHE[key]
    a = np.ascontiguousarray(np.asarray(a, dtype=np.float32))
    b = np.ascontiguousarray(np.asarray(b, dtype=np.float32))
    in_maps = [
        {"a": a[i * NSHARD : (i + 1) * NSHARD], "b": b} for i in range(NCORES)
    ]
    res = run_bass_kernel_spmd(nc, in_maps, list(range(NCORES)), trace=trace)
    out = np.concatenate(
        [np.asarray(r["out"], dtype=np.float32) for r in res.results], axis=0
    )
    return out, res


def kernel(a, b):
    out, _ = _run(a, b, trace=False)
    return out
